# revision 3
# baseline (speedup 1.0000x reference)
"""Trainium2 Bass kernel v3: hybrid waves, 2048-edge macros, minimal in-loop DMAs.

Like v2 (see kernel_v1_backup.py lineage) but: 49 macros of 2048 edges, 7
waves x 7 macros; per trip ONE staging DMA (no interleave - col/row subtiles
are PE-transposed separately into the top/bottom 64 partitions of the PSUM
f12 tile) and ONE out DMA. In-loop DMA executions cost ~0.5-1ms each here,
so per-trip DMA count dominates; this gets 2 per 2048 edges vs v2's 3/1024.
"""

import numpy as np

import concourse.bacc as bacc
import concourse.mybir as mybir
import concourse.tile as tile
import concourse.tile_sem_assignment as _tsa
from concourse.bass_utils import run_bass_kernel_spmd

_orig_assign_tick = _tsa.TileClockTick._assign_tick


def _queue_affine_assign_tick(self, inst):
    if (
        isinstance(inst, _tsa.DMAInst)
        and getattr(inst, "engine", None) == mybir.EngineType.Pool
        and getattr(inst, "queue_num", None) is not None
    ):
        q = inst.queue_num
        tog = getattr(self, "_q_lane_toggle", None)
        if tog is None:
            tog = self._q_lane_toggle = {}
        t = tog.get(q, 0)
        tog[q] = t ^ 1
        self.next_sw_dma_idx = 2 * q + t
    return _orig_assign_tick(self, inst)


_tsa.TileClockTick._assign_tick = _queue_affine_assign_tick

N_NODES = 50000
N_EDGES = 800000
HID = 64
NCORES = 8
EPC = N_EDGES // NCORES           # 100000 edges per core
MAC = 2048                        # edges per macro-tile
NMAC = 49                         # 49*2048 = 100352 -> 352 pad slots
WAVE = 7                          # macros per wave
NWAVE = NMAC // WAVE              # 7
OFF = 32768                       # idx16 = node - OFF
SUB = MAC // 128                  # 16 gather subtiles per endpoint

f32 = mybir.dt.float32
i16 = mybir.dt.int16
relu = mybir.ActivationFunctionType.Relu


def build_nc(repeat: int = 1):
    nc = bacc.Bacc("TRN2", target_bir_lowering=False, debug=False,
                   num_swdge_queues=4)

    embf = nc.dram_tensor("embf", [N_NODES, HID], f32, kind="ExternalInput")
    idxd = nc.dram_tensor("idxd", [128, NMAC, 256], i16, kind="ExternalInput")
    w1d = nc.dram_tensor("w1", [128, 256], f32, kind="ExternalInput")
    w2d = nc.dram_tensor("w2", [128, 2 * HID], f32, kind="ExternalInput")
    w3d = nc.dram_tensor("w3", [HID, 1], f32, kind="ExternalInput")
    b1d = nc.dram_tensor("b1", [128, 2], f32, kind="ExternalInput")
    b2d = nc.dram_tensor("b2", [HID, 1], f32, kind="ExternalInput")
    b3d = nc.dram_tensor("b3", [1, 1], f32, kind="ExternalInput")
    identd = nc.dram_tensor("ident", [128, 128], f32, kind="ExternalInput")
    out = nc.dram_tensor("out", [NMAC, 1, MAC], f32, kind="ExternalOutput")

    with tile.TileContext(nc) as tc:
        with (
            tc.tile_pool(name="const", bufs=1) as cpool,
            tc.tile_pool(name="wave", bufs=2) as wvpool,
            tc.tile_pool(name="act", bufs=2) as apool,
            tc.tile_pool(name="big", bufs=1) as bpool,
            tc.tile_pool(name="ps_t", bufs=1, space="PSUM") as ppool_t,
            tc.tile_pool(name="ps_w", bufs=2, space="PSUM") as ppool_w,
        ):
            ixall = cpool.tile([128, NMAC, 256], i16)
            w1s = cpool.tile([128, 256], f32)
            w2s = cpool.tile([128, 2 * HID], f32)
            w3s = cpool.tile([HID, 1], f32)
            b1s = cpool.tile([128, 2], f32)
            b2s = cpool.tile([HID, 1], f32)
            b3s = cpool.tile([1, 1], f32)
            idn = cpool.tile([128, 128], f32)
            nc.sync.dma_start(ixall[:], idxd[:])
            nc.sync.dma_start(w1s[:], w1d[:])
            nc.sync.dma_start(w2s[:], w2d[:])
            nc.sync.dma_start(w3s[:], w3d[:])
            nc.sync.dma_start(b1s[:], b1d[:])
            nc.sync.dma_start(b2s[:], b2d[:])
            nc.sync.dma_start(b3s[:], b3d[:])
            nc.sync.dma_start(idn[:], identd[:])

            for _rep in range(repeat):
                for w in range(NWAVE):
                    wb = wvpool.tile([128, WAVE, 2 * SUB, HID], f32,
                                     name="wb", tag="wb")
                    for k in range(WAVE):
                        mg = w * WAVE + k
                        nc.gpsimd.dma_gather(
                            wb[:, k, 0:SUB, :], embf[OFF:, :],
                            ixall[:, mg, 0:128], MAC, MAC, HID,
                            transpose=False, queue_num=(2 * k) % 4,
                            single_packet=False)
                        nc.gpsimd.dma_gather(
                            wb[:, k, SUB:2 * SUB, :], embf[OFF:, :],
                            ixall[:, mg, 128:256], MAC, MAC, HID,
                            transpose=False, queue_num=(2 * k + 1) % 4,
                            single_packet=False)

                    with tc.For_i(0, WAVE, 1) as m:
                        # one staged copy (DMA: dyn offsets are io-only),
                        # then static DVE interleave; engine APs are static
                        cr = bpool.tile([128, 2 * SUB, HID], f32, name="cr")
                        nc.sync.dma_start(cr[:], wb[:, m, :, :])
                        ci = bpool.tile([128, 2 * SUB, HID], f32, name="ci")
                        nc.vector.tensor_copy(ci[:, 0::2, :], cr[:, 0:SUB, :])
                        nc.vector.tensor_copy(ci[:, 1::2, :],
                                              cr[:, SUB:2 * SUB, :])
                        tp = ppool_t.tile([128, MAC], f32, name="tp")
                        for c in range(SUB):
                            blk = slice(c * 128, (c + 1) * 128)
                            nc.tensor.transpose(tp[:, blk],
                                                ci[:, 2 * c:2 * c + 2, :],
                                                idn[:])
                        g32 = bpool.tile([128, MAC], f32, name="g32")
                        nc.scalar.copy(g32[:], tp[:])

                        ost = bpool.tile([1, MAC], f32, name="ost")
                        for h in range(2):
                            hof = h * 1024
                            h1a = ppool_w.tile([128, 1024], f32, name="h1a",
                                               tag="work")
                            h1b = ppool_w.tile([128, 1024], f32, name="h1b",
                                               tag="work")
                            for j in range(2):
                                sl = slice(j * 512, (j + 1) * 512)
                                gsl = slice(hof + j * 512, hof + (j + 1) * 512)
                                nc.tensor.matmul(h1a[:, sl], w1s[:, 0:128],
                                                 g32[:, gsl], start=True,
                                                 stop=True)
                                nc.tensor.matmul(h1b[:, sl], w1s[:, 128:256],
                                                 g32[:, gsl], start=True,
                                                 stop=True)
                            s1a = apool.tile([128, 1024], f32, name="s1a")
                            s1b = apool.tile([128, 1024], f32, name="s1b")
                            nc.scalar.activation(s1a[:], h1a[:], relu,
                                                 bias=b1s[:, 0:1])
                            nc.scalar.activation(s1b[:], h1b[:], relu,
                                                 bias=b1s[:, 1:2])

                            h2 = ppool_w.tile([HID, 1024], f32, name="h2",
                                              tag="work")
                            for j in range(2):
                                sl = slice(j * 512, (j + 1) * 512)
                                nc.tensor.matmul(h2[:, sl], w2s[:, 0:HID],
                                                 s1a[:, sl], start=True,
                                                 stop=False)
                                nc.tensor.matmul(h2[:, sl],
                                                 w2s[:, HID:2 * HID],
                                                 s1b[:, sl], start=False,
                                                 stop=True)
                            s2 = apool.tile([HID, 1024], f32, name="s2")
                            nc.scalar.activation(s2[:], h2[:], relu,
                                                 bias=b2s[:])

                            o = ppool_w.tile([1, 1024], f32, name="o",
                                             tag="work")
                            for j in range(2):
                                sl = slice(j * 512, (j + 1) * 512)
                                nc.tensor.matmul(o[:, sl], w3s[:], s2[:, sl],
                                                 start=True, stop=True)
                            nc.vector.tensor_scalar_add(
                                ost[:, hof:hof + 1024], o[:], b3s[0:1, 0:1])
                        nc.sync.dma_start(out[w * WAVE + m, :, :], ost[:])

    nc.compile()
    return nc


def _wrap16(a):
    n = a.shape[0]
    x = a.reshape(n // 16, 16).T.reshape(16, n // 16)
    return np.tile(x, (8, 1)).astype(np.int16)


def prep_inputs(emb, edge_index, W1, b1, W2, b2, W3, b3):
    emb = np.ascontiguousarray(np.asarray(emb, np.float32))
    ei = np.asarray(edge_index).astype(np.int64)
    W1 = np.asarray(W1, np.float32)
    b1 = np.asarray(b1, np.float32)
    W2 = np.asarray(W2, np.float32)
    b2 = np.asarray(b2, np.float32)
    W3 = np.asarray(W3, np.float32)
    b3 = np.asarray(b3, np.float32)

    w2p = np.ascontiguousarray(
        np.concatenate([W2[0:128, :], W2[128:256, :]], axis=1)).astype(np.float32)
    b1p = np.ascontiguousarray(
        np.stack([b1[0:128], b1[128:256]], axis=1)).astype(np.float32)
    ident = np.eye(128, dtype=np.float32)

    in_maps = []
    origpos = []
    for c in range(NCORES):
        sl = slice(c * EPC, (c + 1) * EPC)
        col = ei[0, sl].copy()
        row = ei[1, sl].copy()
        orig = np.arange(c * EPC, (c + 1) * EPC, dtype=np.int64)
        npad = NMAC * MAC - EPC
        col = np.concatenate([col, np.full(npad, N_NODES - 1, np.int64)])
        row = np.concatenate([row, np.full(npad, N_NODES - 1, np.int64)])
        orig = np.concatenate([orig, np.full(npad, -1, np.int64)])

        col2 = col.reshape(NMAC, MAC)
        row2 = row.reshape(NMAC, MAC)
        orig2 = orig.reshape(NMAC, MAC)
        for mv in range(NMAC):
            if col2[mv, -1] >= OFF and row2[mv, -1] >= OFF:
                continue
            cand = np.nonzero((col2[mv] >= OFF) & (row2[mv] >= OFF))[0]
            assert len(cand) > 0, f"core {c} macro {mv}: no high-high edge"
            j = cand[0]
            for arr in (col2, row2, orig2):
                arr[mv, -1], arr[mv, j] = arr[mv, j], arr[mv, -1]

        idx = np.empty((128, NMAC, 256), np.int16)
        for mv in range(NMAC):
            idx[:, mv, 0:128] = _wrap16((col2[mv] - OFF).astype(np.int16))
            idx[:, mv, 128:256] = _wrap16((row2[mv] - OFF).astype(np.int16))

        in_maps.append({
            "embf": emb,
            "idxd": idx,
            "w1": np.ascontiguousarray(W1),
            "w2": w2p,
            "w3": np.ascontiguousarray(W3),
            "b1": b1p,
            "b2": np.ascontiguousarray(b2[:, None]),
            "b3": b3.reshape(1, 1),
            "ident": ident,
        })
        origpos.append(orig2.reshape(-1))
    return in_maps, origpos


def unshard(results, origpos):
    out_full = np.empty((N_EDGES, 1), np.float32)
    for c in range(NCORES):
        vals = results[c]["out"].reshape(-1)
        orig = origpos[c]
        valid = orig >= 0
        out_full[orig[valid], 0] = vals[valid]
    return out_full


_NC_CACHE = {}


def _get_nc(repeat: int = 1):
    if repeat not in _NC_CACHE:
        _NC_CACHE[repeat] = build_nc(repeat)
    return _NC_CACHE[repeat]


def kernel(**inputs) -> np.ndarray:
    nc = _get_nc(1)
    in_maps, origpos = prep_inputs(
        inputs["emb"], inputs["edge_index"],
        inputs["W1"], inputs["b1"], inputs["W2"], inputs["b2"],
        inputs["W3"], inputs["b3"])
    res = run_bass_kernel_spmd(nc, in_maps, core_ids=list(range(NCORES)))
    return unshard(res.results, origpos)


# revision 4
# speedup vs baseline: 4.3127x; 4.3127x over previous
"""Trainium2 Bass kernel: hybrid waves, 2048-edge macros, 2-macro loop trips.

50 macros of 2048 edges in 5 waves of 10; unrolled gathers fill a wave
buffer (gathers inside For_i cost ~18ms/trip flat, so they stay out of
loops), then each For_i trip stages TWO macros with one dynamic-offset DMA
(engine instructions cannot take loop-var AP offsets - NEFF disables
vector_dynamic_offsets - so dynamic reads go through DMA into fixed tiles),
interleaves via static DVE copies, PE-transposes to feature-major, runs the
3-layer MLP on PE/ACT, and writes one [1,4096] out DMA. In-loop DMA
executions dominate the dynamic cost (~0.5-1ms each); this structure has 2
per 4096 edges -> ~34-48ms/repeat measured (baseline: 300-400ms).
"""

import numpy as np

import concourse.bacc as bacc
import concourse.mybir as mybir
import concourse.tile as tile
import concourse.tile_sem_assignment as _tsa
from concourse.bass_utils import run_bass_kernel_spmd

_orig_assign_tick = _tsa.TileClockTick._assign_tick


def _queue_affine_assign_tick(self, inst):
    if (
        isinstance(inst, _tsa.DMAInst)
        and getattr(inst, "engine", None) == mybir.EngineType.Pool
        and getattr(inst, "queue_num", None) is not None
    ):
        q = inst.queue_num
        tog = getattr(self, "_q_lane_toggle", None)
        if tog is None:
            tog = self._q_lane_toggle = {}
        t = tog.get(q, 0)
        tog[q] = t ^ 1
        self.next_sw_dma_idx = 2 * q + t
    return _orig_assign_tick(self, inst)


_tsa.TileClockTick._assign_tick = _queue_affine_assign_tick

N_NODES = 50000
N_EDGES = 800000
HID = 64
NCORES = 8
EPC = N_EDGES // NCORES           # 100000 edges per core
MAC = 2048                        # edges per macro-tile
NMAC = 50                         # 50*2048 = 102400 -> 2400 pad slots
WAVE = 10                         # macros per wave
NWAVE = NMAC // WAVE              # 5
OFF = 32768                       # idx16 = node - OFF
SUB = MAC // 128                  # 16 gather subtiles per endpoint

f32 = mybir.dt.float32
i16 = mybir.dt.int16
relu = mybir.ActivationFunctionType.Relu


def build_nc(repeat: int = 1):
    nc = bacc.Bacc("TRN2", target_bir_lowering=False, debug=False,
                   num_swdge_queues=4)

    embf = nc.dram_tensor("embf", [N_NODES, HID], f32, kind="ExternalInput")
    idxd = nc.dram_tensor("idxd", [128, NMAC, 256], i16, kind="ExternalInput")
    w1d = nc.dram_tensor("w1", [128, 256], f32, kind="ExternalInput")
    w2d = nc.dram_tensor("w2", [128, 2 * HID], f32, kind="ExternalInput")
    w3d = nc.dram_tensor("w3", [HID, 1], f32, kind="ExternalInput")
    b1d = nc.dram_tensor("b1", [128, 2], f32, kind="ExternalInput")
    b2d = nc.dram_tensor("b2", [HID, 1], f32, kind="ExternalInput")
    b3d = nc.dram_tensor("b3", [1, 1], f32, kind="ExternalInput")
    identd = nc.dram_tensor("ident", [128, 128], f32, kind="ExternalInput")
    out = nc.dram_tensor("out", [NMAC, 1, MAC], f32, kind="ExternalOutput")

    with tile.TileContext(nc) as tc:
        with (
            tc.tile_pool(name="const", bufs=1) as cpool,
            tc.tile_pool(name="wave", bufs=1) as wvpool,
            tc.tile_pool(name="act", bufs=2) as apool,
            tc.tile_pool(name="big", bufs=1) as bpool,
            tc.tile_pool(name="ps_t", bufs=1, space="PSUM") as ppool_t,
            tc.tile_pool(name="ps_w", bufs=2, space="PSUM") as ppool_w,
        ):
            ixall = cpool.tile([128, NMAC, 256], i16)
            w1s = cpool.tile([128, 256], f32)
            w2s = cpool.tile([128, 2 * HID], f32)
            w3s = cpool.tile([HID, 1], f32)
            b1s = cpool.tile([128, 2], f32)
            b2s = cpool.tile([HID, 1], f32)
            b3s = cpool.tile([1, 1], f32)
            idn = cpool.tile([128, 128], f32)
            nc.sync.dma_start(ixall[:], idxd[:])
            nc.sync.dma_start(w1s[:], w1d[:])
            nc.sync.dma_start(w2s[:], w2d[:])
            nc.sync.dma_start(w3s[:], w3d[:])
            nc.sync.dma_start(b1s[:], b1d[:])
            nc.sync.dma_start(b2s[:], b2d[:])
            nc.sync.dma_start(b3s[:], b3d[:])
            nc.sync.dma_start(idn[:], identd[:])

            for _rep in range(repeat):
                for w in range(NWAVE):
                    wb = wvpool.tile([128, WAVE, 2 * SUB, HID], f32,
                                     name="wb", tag="wb")
                    for k in range(WAVE):
                        mg = w * WAVE + k
                        nc.gpsimd.dma_gather(
                            wb[:, k, 0:SUB, :], embf[OFF:, :],
                            ixall[:, mg, 0:128], MAC, MAC, HID,
                            transpose=False, queue_num=(2 * k) % 4,
                            single_packet=False)
                        nc.gpsimd.dma_gather(
                            wb[:, k, SUB:2 * SUB, :], embf[OFF:, :],
                            ixall[:, mg, 128:256], MAC, MAC, HID,
                            transpose=False, queue_num=(2 * k + 1) % 4,
                            single_packet=False)

                    wbp = wb.rearrange("p (t u) s h -> p t (u s) h", u=2)
                    outp = out.rearrange("(t u) o e -> t o (u e)", u=2)
                    with tc.For_i(0, WAVE // 2, 1) as m:
                        # one staged copy of TWO macros (DMA: dyn offsets are
                        # io-only), then static DVE interleave per macro
                        cr = bpool.tile([128, 4 * SUB, HID], f32, name="cr")
                        nc.sync.dma_start(cr[:], wbp[:, m, :, :])
                        ost2 = bpool.tile([1, 2 * MAC], f32, name="ost2")
                        for u in range(2):
                          uo = u * 2 * SUB
                          if True:
                            ci = bpool.tile([128, 2 * SUB, HID], f32,
                                            name="ci", tag="ci")
                            nc.vector.tensor_copy(ci[:, 0::2, :],
                                                  cr[:, uo:uo + SUB, :])
                            nc.vector.tensor_copy(
                                ci[:, 1::2, :],
                                cr[:, uo + SUB:uo + 2 * SUB, :])
                            tp = ppool_t.tile([128, MAC], f32, name="tp",
                                              tag="tp")
                            for c in range(SUB):
                                blk = slice(c * 128, (c + 1) * 128)
                                nc.tensor.transpose(tp[:, blk],
                                                    ci[:, 2 * c:2 * c + 2, :],
                                                    idn[:])
                            g32 = bpool.tile([128, MAC], f32, name="g32",
                                             tag="g32")
                            nc.scalar.copy(g32[:], tp[:])

                            ost = ost2[:, u * MAC:(u + 1) * MAC]
                            for h in range(2):
                                hof = h * 1024
                                h1a = ppool_w.tile([128, 1024], f32,
                                                   name="h1a", tag="work")
                                h1b = ppool_w.tile([128, 1024], f32,
                                                   name="h1b", tag="work")
                                for j in range(2):
                                    sl = slice(j * 512, (j + 1) * 512)
                                    gsl = slice(hof + j * 512,
                                                hof + (j + 1) * 512)
                                    nc.tensor.matmul(h1a[:, sl],
                                                     w1s[:, 0:128],
                                                     g32[:, gsl], start=True,
                                                     stop=True)
                                    nc.tensor.matmul(h1b[:, sl],
                                                     w1s[:, 128:256],
                                                     g32[:, gsl], start=True,
                                                     stop=True)
                                s1a = apool.tile([128, 1024], f32, name="s1a")
                                s1b = apool.tile([128, 1024], f32, name="s1b")
                                nc.scalar.activation(s1a[:], h1a[:], relu,
                                                     bias=b1s[:, 0:1])
                                nc.scalar.activation(s1b[:], h1b[:], relu,
                                                     bias=b1s[:, 1:2])

                                h2 = ppool_w.tile([HID, 1024], f32, name="h2",
                                                  tag="work")
                                for j in range(2):
                                    sl = slice(j * 512, (j + 1) * 512)
                                    nc.tensor.matmul(h2[:, sl], w2s[:, 0:HID],
                                                     s1a[:, sl], start=True,
                                                     stop=False)
                                    nc.tensor.matmul(h2[:, sl],
                                                     w2s[:, HID:2 * HID],
                                                     s1b[:, sl], start=False,
                                                     stop=True)
                                s2 = apool.tile([HID, 1024], f32, name="s2")
                                nc.scalar.activation(s2[:], h2[:], relu,
                                                     bias=b2s[:])

                                o = ppool_w.tile([1, 1024], f32, name="o",
                                                 tag="work")
                                for j in range(2):
                                    sl = slice(j * 512, (j + 1) * 512)
                                    nc.tensor.matmul(o[:, sl], w3s[:],
                                                     s2[:, sl], start=True,
                                                     stop=True)
                                nc.vector.tensor_scalar_add(
                                    ost[:, hof:hof + 1024], o[:],
                                    b3s[0:1, 0:1])
                        nc.sync.dma_start(outp[w * (WAVE // 2) + m, :, :],
                                          ost2[:])

    nc.compile()
    return nc


def _wrap16(a):
    n = a.shape[0]
    x = a.reshape(n // 16, 16).T.reshape(16, n // 16)
    return np.tile(x, (8, 1)).astype(np.int16)


def prep_inputs(emb, edge_index, W1, b1, W2, b2, W3, b3):
    emb = np.ascontiguousarray(np.asarray(emb, np.float32))
    ei = np.asarray(edge_index).astype(np.int64)
    W1 = np.asarray(W1, np.float32)
    b1 = np.asarray(b1, np.float32)
    W2 = np.asarray(W2, np.float32)
    b2 = np.asarray(b2, np.float32)
    W3 = np.asarray(W3, np.float32)
    b3 = np.asarray(b3, np.float32)

    w2p = np.ascontiguousarray(
        np.concatenate([W2[0:128, :], W2[128:256, :]], axis=1)).astype(np.float32)
    b1p = np.ascontiguousarray(
        np.stack([b1[0:128], b1[128:256]], axis=1)).astype(np.float32)
    ident = np.eye(128, dtype=np.float32)

    in_maps = []
    origpos = []
    for c in range(NCORES):
        sl = slice(c * EPC, (c + 1) * EPC)
        col = ei[0, sl].copy()
        row = ei[1, sl].copy()
        orig = np.arange(c * EPC, (c + 1) * EPC, dtype=np.int64)
        npad = NMAC * MAC - EPC
        col = np.concatenate([col, np.full(npad, N_NODES - 1, np.int64)])
        row = np.concatenate([row, np.full(npad, N_NODES - 1, np.int64)])
        orig = np.concatenate([orig, np.full(npad, -1, np.int64)])

        col2 = col.reshape(NMAC, MAC)
        row2 = row.reshape(NMAC, MAC)
        orig2 = orig.reshape(NMAC, MAC)
        for mv in range(NMAC):
            if col2[mv, -1] >= OFF and row2[mv, -1] >= OFF:
                continue
            cand = np.nonzero((col2[mv] >= OFF) & (row2[mv] >= OFF))[0]
            assert len(cand) > 0, f"core {c} macro {mv}: no high-high edge"
            j = cand[0]
            for arr in (col2, row2, orig2):
                arr[mv, -1], arr[mv, j] = arr[mv, j], arr[mv, -1]

        idx = np.empty((128, NMAC, 256), np.int16)
        for mv in range(NMAC):
            idx[:, mv, 0:128] = _wrap16((col2[mv] - OFF).astype(np.int16))
            idx[:, mv, 128:256] = _wrap16((row2[mv] - OFF).astype(np.int16))

        in_maps.append({
            "embf": emb,
            "idxd": idx,
            "w1": np.ascontiguousarray(W1),
            "w2": w2p,
            "w3": np.ascontiguousarray(W3),
            "b1": b1p,
            "b2": np.ascontiguousarray(b2[:, None]),
            "b3": b3.reshape(1, 1),
            "ident": ident,
        })
        origpos.append(orig2.reshape(-1))
    return in_maps, origpos


def unshard(results, origpos):
    out_full = np.empty((N_EDGES, 1), np.float32)
    for c in range(NCORES):
        vals = results[c]["out"].reshape(-1)
        orig = origpos[c]
        valid = orig >= 0
        out_full[orig[valid], 0] = vals[valid]
    return out_full


_NC_CACHE = {}


def _get_nc(repeat: int = 1):
    if repeat not in _NC_CACHE:
        _NC_CACHE[repeat] = build_nc(repeat)
    return _NC_CACHE[repeat]


def kernel(**inputs) -> np.ndarray:
    nc = _get_nc(1)
    in_maps, origpos = prep_inputs(
        inputs["emb"], inputs["edge_index"],
        inputs["W1"], inputs["b1"], inputs["W2"], inputs["b2"],
        inputs["W3"], inputs["b3"])
    res = run_bass_kernel_spmd(nc, in_maps, core_ids=list(range(NCORES)))
    return unshard(res.results, origpos)
